# revision 9
# baseline (speedup 1.0000x reference)
"""Weighted-BCE loss on Trainium2, data-parallel over 8 NeuronCores.

Strategy
--------
Shard the batch dim 8 ways (125k rows / core); each shard is a flat stream of
2,875,000 f32 elements with channel id = flat_idx % 23 (all tile offsets and
matmul widths are kept == 0 mod 23, so per-channel weights apply after the
reductions on tiny [1,506] vectors).

Single-Ln restructure: with a' = x + t - 1 (t = label in {0,1}):
  sign(a') encodes t, |a'| = t?x:(1-x), and
  Ln|a'| = t*ln(x) + (1-t)*ln(1-x)   -- exactly one BCE branch per element.
Per channel c we accumulate A[c] = sum t*ln(x) and B[c] = sum (1-t)*ln(1-x);
loss = -(1/BC) sum_c [a1[c]*A[c] + a0[c]*B[c]],  a1 = 1/w1, a0 = 1/w0.

Per [128, 4048] chunk (both inputs stream through SWDGE with dtype casts in
the DMA datapath -- f32->bf16 for x, int32->bf16 for labels -- which measured
substantially faster than the documented HWDGE path here):
  DVE STT  : a' = (x + (-1)) + t          (one fused two-ALU op, bf16 2x)
Then chunks alternate between two formulations (mix "AMAMAM") to BALANCE the
ACT and DVE engines -- ACT passes cost ~2x a DVE pass, so neither all-1-Ln
nor all-2-Ln is optimal:
  A-chunk (1 ACT + 2 DVE):
    DVE TS : s = a' & 0x7fff  (uint16 view: clears sign bit -> |a'|, 4x mode)
    ACT    : L = Ln(s)            -> column-sums pL
    DVE TT : u = t * L            -> column-sums pU        (A=pU, B=pL-pU)
  M-chunk (2 ACT + 2 cheap DVE):
    ACT    : h1 = Ln(a')  -- finite exactly where t=1, NaN where t=0
    ACT    : h2 = Ln(-a') -- finite exactly where t=0, NaN where t=1
    DVE TS : u1 = min(h1, 0), u2 = min(h2, 0)  -- DVE min flushes NaN to 0,
             so the t-masking is free   -> column-sums pA (=A), pB (=B)
  PE       : ones-vector matmuls into [1,506] f32 PSUM accumulators
The final combine folds each [1,506] PSUM vector to [1,23] with a strided
reduce, applies the per-channel weight patterns, and DMAs one f32 scalar per
core; the host adds the 8 partials (the all-reduce of the sharding hint).
"""
import math
from contextlib import ExitStack

import numpy as np

import concourse.bacc as bacc
import concourse.tile as tile
from concourse import mybir
from concourse import bass_utils

# ---- problem constants (must match the grading harness) ----
B, C = 1_000_000, 23
N_CORES = 8
ROWS_PER_CORE = B // N_CORES
N_ELEMS = ROWS_PER_CORE * C  # flat f32 elements per core

P = 128
F_LOAD = 4048  # load-tile free dim: 176*23 (2 MiB f32 reads per x-DMA);
               # bigger 8096 load tiles measured 2x SLOWER (coarser DMA->compute
               # sync granularity starves the engines)
F_COMP = 4048  # compute-slice free dim: 176*23
MM_W = 506     # matmul free width: 22*23, <= 512 (one fp32 PSUM bank)
MIX = "AMAMAM"  # per-slice formulation; balances ACT vs DVE engine time

_W = np.array(
    [0.0012597430655963838, 0.0004919313290455535, 0.0021106513104319356,
     0.0007678117365508301, 0.004719881670572202, 0.000372272357115554,
     0.029090425620315438, 0.010056339432617042, 0.0034817436971298467,
     0.0003057951504877765, 0.003995280118329428, 8.808229878180519e-05,
     0.012070598793438699, 0.016788818533845208, 0.0017832510677901316,
     0.0008758371973209686, 0.0005933090691529143, 0.0031992155689617922,
     0.003212511010287348, 0.0016685778863572154, 0.0009356666832859684,
     0.0010985358395240233, 0.00103372056306194], dtype=np.float32)

# mirror the reference's f32 arithmetic exactly
_WEIGHT_0 = (1.0 / (_W + 1.0)).astype(np.float32)   # used when target == 0
_WEIGHT_1 = (1.0 - _WEIGHT_0).astype(np.float32)    # used when target == 1
_A0 = (np.float32(1.0) / _WEIGHT_0).astype(np.float64)
_A1 = (np.float32(1.0) / _WEIGHT_1).astype(np.float64)
_SCALE = 1.0 / (float(B) * float(C))

# loss = sum_f PATU*pU + PATL*pL  (A-chunks)  +  PATA*pA + PATL*pB  (M-chunks)
PATU = ((_A0 - _A1) * _SCALE).astype(np.float32).reshape(1, C)
PATL = (-_A0 * _SCALE).astype(np.float32).reshape(1, C)
PATA = (-_A1 * _SCALE).astype(np.float32).reshape(1, C)


def _plan_chunks(n_elems, p=P, f_full=F_LOAD):
    """Cover the flat stream with [p, f] tiles, all offsets/strides = 0 mod C."""
    assert f_full % C == 0
    tile_elems = p * f_full
    chunks = []
    off = 0
    while n_elems - off >= tile_elems:
        chunks.append((off, p, f_full))
        off += tile_elems
    r = n_elems - off
    if r:
        assert r % C == 0, "tail must stay channel-aligned"
        m = r // C
        for pp in range(min(p, m), 0, -1):
            if m % pp == 0 and C * (m // pp) <= 2 * f_full:
                ff = C * (m // pp)
                break
        else:
            raise ValueError(f"cannot tile tail of {r} elements")
        chunks.append((off, pp, ff))
    return chunks


def _plan_slices(chunks, mix=None):
    """Split each load chunk into <=F_COMP column slices; assign A/M types."""
    slices = []
    k = 0
    for ci, (off, p, f) in enumerate(chunks):
        j0 = 0
        while j0 < f:
            fs = min(F_COMP, f - j0)
            assert fs % C == 0
            ty = (mix or MIX)[k % len(mix or MIX)]
            slices.append((ci, j0, fs, ty))
            j0 += fs
            k += 1
    return slices


def build_bass(repeat=1, num_devices=N_CORES, n_elems=N_ELEMS, mix=None,
               io_bufs=3, wk_bufs=2, stt_gpsimd_m=False, psum_pad=False):
    f32 = mybir.dt.float32
    bf16 = mybir.dt.bfloat16
    i32 = mybir.dt.int32
    u16 = mybir.dt.uint16
    Ln = mybir.ActivationFunctionType.Ln
    add = mybir.AluOpType.add
    mult = mybir.AluOpType.mult
    band = mybir.AluOpType.bitwise_and

    nc = bacc.Bacc("TRN2", target_bir_lowering=False, debug=False,
                   enable_asserts=False, num_devices=num_devices)

    x_d = nc.dram_tensor("x", [n_elems], f32, kind="ExternalInput").ap()
    l_d = nc.dram_tensor("lab", [n_elems], i32, kind="ExternalInput").ap()
    patu_d = nc.dram_tensor("patu", [1, C], f32, kind="ExternalInput").ap()
    patl_d = nc.dram_tensor("patl", [1, C], f32, kind="ExternalInput").ap()
    pata_d = nc.dram_tensor("pata", [1, C], f32, kind="ExternalInput").ap()
    out_d = nc.dram_tensor("out", [1, 1], f32, kind="ExternalOutput").ap()

    chunks = _plan_chunks(n_elems, P, F_LOAD)
    slices = _plan_slices(chunks, mix)
    f_alloc = max(f for _, _, f in chunks)
    fs_alloc = max(fs for _, _, fs, _ in slices)
    n_mm_a = repeat * sum(-(-fs // MM_W) for _, _, fs, ty in slices if ty == "A")
    n_mm_m = repeat * sum(-(-fs // MM_W) for _, _, fs, ty in slices if ty == "M")

    with tile.TileContext(nc) as tc, ExitStack() as ctx:
        io = ctx.enter_context(tc.tile_pool(name="io", bufs=io_bufs))
        wk = ctx.enter_context(tc.tile_pool(name="wk", bufs=wk_bufs))
        sg = ctx.enter_context(tc.tile_pool(name="sg", bufs=1))
        ps = ctx.enter_context(tc.tile_pool(name="ps", bufs=1, space="PSUM"))

        ones = sg.tile([P, 1], bf16, tag="ones")
        nc.vector.memset(ones, 1.0)
        patu_t = sg.tile([1, C], f32, tag="patu")
        patl_t = sg.tile([1, C], f32, tag="patl")
        pata_t = sg.tile([1, C], f32, tag="pata")
        # consts ride SWDGE too; they are tiny and go first
        nc.gpsimd.dma_start(out=patu_t, in_=patu_d)
        nc.gpsimd.dma_start(out=patl_t, in_=patl_d)
        nc.gpsimd.dma_start(out=pata_t, in_=pata_d)

        # psum_pad: allocate 512-wide (2048 B = exactly one PSUM bank) so no
        # accumulator straddles a bank boundary
        pw = 512 if psum_pad else MM_W
        pL_full = ps.tile([1, pw], f32, tag="pL", name="pL_full")
        pU_full = ps.tile([1, pw], f32, tag="pU", name="pU_full")
        pA_full = ps.tile([1, pw], f32, tag="pA", name="pA_full")
        pB_full = ps.tile([1, pw], f32, tag="pB", name="pB_full")
        pL = pL_full[:, :MM_W]
        pU = pU_full[:, :MM_W]
        pA = pA_full[:, :MM_W]
        pB = pB_full[:, :MM_W]

        mma = 0
        mmm = 0
        for _ in range(repeat):
            tiles = {}
            for ci, (off, p, f) in enumerate(chunks):
                src_x = x_d[off:off + p * f].rearrange("(p f) -> p f", f=f)
                src_l = l_d[off:off + p * f].rearrange("(p f) -> p f", f=f)
                xt = io.tile([P, f_alloc], bf16, tag="xt", name="xt")
                tb = io.tile([P, f_alloc], bf16, tag="tb", name="tb")
                nc.gpsimd.dma_start(out=xt[:p, :f], in_=src_x)
                nc.gpsimd.dma_start(out=tb[:p, :f], in_=src_l)
                tiles[ci] = (xt, tb, p)
                # compute on the <=F_COMP column slices of this load tile
                for sci, j0, fs, ty in slices:
                    if sci != ci:
                        continue
                    xs = xt[:p, j0:j0 + fs]
                    ts = tb[:p, j0:j0 + fs]
                    ap_t = wk.tile([P, fs_alloc], bf16, tag="ap", name="ap_t")
                    stt_eng = nc.gpsimd if (stt_gpsimd_m and ty == "M") else nc.vector
                    stt_eng.scalar_tensor_tensor(ap_t[:p, :fs], xs, -1.0,
                                                 ts, add, add)
                    if ty == "A":
                        s_t = wk.tile([P, fs_alloc], bf16, tag="w1", name="s_t")
                        L_t = wk.tile([P, fs_alloc], bf16, tag="w2", name="L_t")
                        u_t = wk.tile([P, fs_alloc], bf16, tag="w3", name="u_t")
                        nc.vector.tensor_scalar(s_t[:p, :fs].bitcast(u16),
                                                ap_t[:p, :fs].bitcast(u16),
                                                0x7FFF, None, band)
                        nc.scalar.activation(L_t[:p, :fs], s_t[:p, :fs], Ln)
                        nc.vector.tensor_tensor(u_t[:p, :fs], ts, L_t[:p, :fs],
                                                mult)
                        for j in range(0, fs, MM_W):
                            wd = min(MM_W, fs - j)
                            nc.tensor.matmul(pL[:, :wd], ones[:p, :],
                                             L_t[:p, j:j + wd],
                                             start=(mma == 0),
                                             stop=(mma == n_mm_a - 1))
                            nc.tensor.matmul(pU[:, :wd], ones[:p, :],
                                             u_t[:p, j:j + wd],
                                             start=(mma == 0),
                                             stop=(mma == n_mm_a - 1))
                            mma += 1
                    else:
                        h1_t = wk.tile([P, fs_alloc], bf16, tag="w1", name="h1_t")
                        h2_t = wk.tile([P, fs_alloc], bf16, tag="w2", name="h2_t")
                        u1_t = wk.tile([P, fs_alloc], bf16, tag="w3", name="u1_t")
                        u2_t = wk.tile([P, fs_alloc], bf16, tag="w4", name="u2_t")
                        nc.scalar.activation(h1_t[:p, :fs], ap_t[:p, :fs], Ln)
                        nc.scalar.activation(h2_t[:p, :fs], ap_t[:p, :fs], Ln,
                                             scale=-1.0)
                        nc.vector.tensor_scalar_min(u1_t[:p, :fs],
                                                    h1_t[:p, :fs], 0.0)
                        nc.vector.tensor_scalar_min(u2_t[:p, :fs],
                                                    h2_t[:p, :fs], 0.0)
                        for j in range(0, fs, MM_W):
                            wd = min(MM_W, fs - j)
                            nc.tensor.matmul(pA[:, :wd], ones[:p, :],
                                             u1_t[:p, j:j + wd],
                                             start=(mmm == 0),
                                             stop=(mmm == n_mm_m - 1))
                            nc.tensor.matmul(pB[:, :wd], ones[:p, :],
                                             u2_t[:p, j:j + wd],
                                             start=(mmm == 0),
                                             stop=(mmm == n_mm_m - 1))
                            mmm += 1

        # final fold: strided reduce [1,506]->[1,C], weight, sum -> scalar
        parts = []
        if n_mm_a:
            cu = sg.tile([1, C], f32, tag="cu")
            cl = sg.tile([1, C], f32, tag="cl")
            nc.vector.reduce_sum(cu, pU.rearrange("one (r c) -> one c r", c=C),
                                 axis=mybir.AxisListType.X)
            nc.vector.reduce_sum(cl, pL.rearrange("one (r c) -> one c r", c=C),
                                 axis=mybir.AxisListType.X)
            s1 = sg.tile([1, C], f32, tag="s1")
            s2 = sg.tile([1, C], f32, tag="s2")
            nc.vector.tensor_mul(s1, cu, patu_t)
            nc.vector.tensor_mul(s2, cl, patl_t)
            nc.vector.tensor_add(s1, s1, s2)
            parts.append(s1)
        if n_mm_m:
            ca = sg.tile([1, C], f32, tag="ca")
            cb = sg.tile([1, C], f32, tag="cb")
            nc.vector.reduce_sum(ca, pA.rearrange("one (r c) -> one c r", c=C),
                                 axis=mybir.AxisListType.X)
            nc.vector.reduce_sum(cb, pB.rearrange("one (r c) -> one c r", c=C),
                                 axis=mybir.AxisListType.X)
            s3 = sg.tile([1, C], f32, tag="s3")
            s4 = sg.tile([1, C], f32, tag="s4")
            nc.vector.tensor_mul(s3, ca, pata_t)
            nc.vector.tensor_mul(s4, cb, patl_t)
            nc.vector.tensor_add(s3, s3, s4)
            parts.append(s3)
        tot = parts[0]
        if len(parts) == 2:
            nc.vector.tensor_add(tot, tot, parts[1])
        accf = sg.tile([1, 1], f32, tag="accf")
        nc.vector.reduce_sum(accf, tot, axis=mybir.AxisListType.X)
        nc.sync.dma_start(out=out_d, in_=accf)

    nc.compile()
    return nc


_CACHE = {}


def _get_nc():
    if "nc" not in _CACHE:
        _CACHE["nc"] = build_bass()
    return _CACHE["nc"]


def kernel(x, labels):
    x = np.ascontiguousarray(np.asarray(x, dtype=np.float32))
    labels = np.ascontiguousarray(np.asarray(labels, dtype=np.int32))
    assert x.shape == (B, C), x.shape
    assert labels.shape == (B, C), labels.shape

    nc = _get_nc()
    in_maps = []
    for i in range(N_CORES):
        sl = slice(i * ROWS_PER_CORE, (i + 1) * ROWS_PER_CORE)
        in_maps.append({
            "x": np.ascontiguousarray(x[sl]).reshape(-1),
            "lab": np.ascontiguousarray(labels[sl]).reshape(-1),
            "patu": PATU,
            "patl": PATL,
            "pata": PATA,
        })
    res = bass_utils.run_bass_kernel_spmd(nc, in_maps, core_ids=list(range(N_CORES)))
    total = 0.0
    for r in res.results:
        total += float(r["out"][0, 0])
    return np.float32(total)


# revision 10
# speedup vs baseline: 1.4242x; 1.4242x over previous
"""Weighted-BCE loss on Trainium2, data-parallel over 8 NeuronCores.

Strategy
--------
Shard the batch dim 8 ways (125k rows / core); each shard is a flat stream of
2,875,000 f32 elements with channel id = flat_idx % 23 (all tile offsets and
matmul widths are kept == 0 mod 23, so per-channel weights apply after the
reductions on tiny [1,506] vectors).

Single-Ln restructure: with a' = x + t - 1 (t = label in {0,1}):
  sign(a') encodes t, |a'| = t?x:(1-x), and
  Ln|a'| = t*ln(x) + (1-t)*ln(1-x)   -- exactly one BCE branch per element.
Per channel c we accumulate A[c] = sum t*ln(x) and B[c] = sum (1-t)*ln(1-x);
loss = -(1/BC) sum_c [a1[c]*A[c] + a0[c]*B[c]],  a1 = 1/w1, a0 = 1/w0.

Per [128, 4048] chunk (both inputs stream through SWDGE with dtype casts in
the DMA datapath -- f32->bf16 for x, int32->bf16 for labels -- which measured
substantially faster than the documented HWDGE path here):
  DVE STT  : a' = (x + (-1)) + t          (one fused two-ALU op, bf16 2x)
Then chunks alternate between two formulations (mix "AMAMAM") to BALANCE the
ACT and DVE engines -- ACT passes cost ~2x a DVE pass, so neither all-1-Ln
nor all-2-Ln is optimal:
  A-chunk (1 ACT + 2 DVE):
    DVE TS : s = a' & 0x7fff  (uint16 view: clears sign bit -> |a'|, 4x mode)
    ACT    : L = Ln(s)            -> column-sums pL
    DVE TT : u = t * L            -> column-sums pU        (A=pU, B=pL-pU)
  M-chunk (2 ACT + 2 cheap DVE):
    ACT    : h1 = Ln(a')  -- finite exactly where t=1, NaN where t=0
    ACT    : h2 = Ln(-a') -- finite exactly where t=0, NaN where t=1
    DVE TS : u1 = min(h1, 0), u2 = min(h2, 0)  -- DVE min flushes NaN to 0,
             so the t-masking is free   -> column-sums pA (=A), pB (=B)
  PE       : ones-vector matmuls into [1,506] f32 PSUM accumulators
The final combine folds each [1,506] PSUM vector to [1,23] with a strided
reduce, applies the per-channel weight patterns, and DMAs one f32 scalar per
core; the host adds the 8 partials (the all-reduce of the sharding hint).
"""
import math
from contextlib import ExitStack

import numpy as np

import concourse.bacc as bacc
import concourse.tile as tile
from concourse import mybir
from concourse import bass_utils

# ---- problem constants (must match the grading harness) ----
B, C = 1_000_000, 23
N_CORES = 8
ROWS_PER_CORE = B // N_CORES
N_ELEMS = ROWS_PER_CORE * C  # flat f32 elements per core

P = 128
F_LOAD = 4048  # load-tile free dim: 176*23 (2 MiB f32 reads per x-DMA);
               # bigger 8096 load tiles measured 2x SLOWER (coarser DMA->compute
               # sync granularity starves the engines)
F_COMP = 4048  # compute-slice free dim: 176*23
MM_W = 506     # matmul free width: 22*23, <= 512 (one fp32 PSUM bank)
MIX = "AMAMAM"  # per-slice formulation; balances ACT vs DVE engine time

_W = np.array(
    [0.0012597430655963838, 0.0004919313290455535, 0.0021106513104319356,
     0.0007678117365508301, 0.004719881670572202, 0.000372272357115554,
     0.029090425620315438, 0.010056339432617042, 0.0034817436971298467,
     0.0003057951504877765, 0.003995280118329428, 8.808229878180519e-05,
     0.012070598793438699, 0.016788818533845208, 0.0017832510677901316,
     0.0008758371973209686, 0.0005933090691529143, 0.0031992155689617922,
     0.003212511010287348, 0.0016685778863572154, 0.0009356666832859684,
     0.0010985358395240233, 0.00103372056306194], dtype=np.float32)

# mirror the reference's f32 arithmetic exactly
_WEIGHT_0 = (1.0 / (_W + 1.0)).astype(np.float32)   # used when target == 0
_WEIGHT_1 = (1.0 - _WEIGHT_0).astype(np.float32)    # used when target == 1
_A0 = (np.float32(1.0) / _WEIGHT_0).astype(np.float64)
_A1 = (np.float32(1.0) / _WEIGHT_1).astype(np.float64)
_SCALE = 1.0 / (float(B) * float(C))

# loss = sum_f PATU*pU + PATL*pL  (A-chunks)  +  PATA*pA + PATL*pB  (M-chunks)
PATU = ((_A0 - _A1) * _SCALE).astype(np.float32).reshape(1, C)
PATL = (-_A0 * _SCALE).astype(np.float32).reshape(1, C)
PATA = (-_A1 * _SCALE).astype(np.float32).reshape(1, C)


def _plan_chunks(n_elems, p=P, f_full=F_LOAD):
    """Cover the flat stream with [p, f] tiles, all offsets/strides = 0 mod C."""
    assert f_full % C == 0
    tile_elems = p * f_full
    chunks = []
    off = 0
    while n_elems - off >= tile_elems:
        chunks.append((off, p, f_full))
        off += tile_elems
    r = n_elems - off
    if r:
        assert r % C == 0, "tail must stay channel-aligned"
        m = r // C
        for pp in range(min(p, m), 0, -1):
            if m % pp == 0 and C * (m // pp) <= 2 * f_full:
                ff = C * (m // pp)
                break
        else:
            raise ValueError(f"cannot tile tail of {r} elements")
        chunks.append((off, pp, ff))
    return chunks


def _plan_slices(chunks, mix=None):
    """Split each load chunk into <=F_COMP column slices; assign A/M types."""
    slices = []
    k = 0
    for ci, (off, p, f) in enumerate(chunks):
        j0 = 0
        while j0 < f:
            fs = min(F_COMP, f - j0)
            assert fs % C == 0
            ty = (mix or MIX)[k % len(mix or MIX)]
            slices.append((ci, j0, fs, ty))
            j0 += fs
            k += 1
    return slices


def build_bass(repeat=1, num_devices=N_CORES, n_elems=N_ELEMS, mix=None,
               io_bufs=3, wk_bufs=2, stt_gpsimd_m=False, psum_pad=False,
               g_from_sign=False):
    f32 = mybir.dt.float32
    bf16 = mybir.dt.bfloat16
    i32 = mybir.dt.int32
    u16 = mybir.dt.uint16
    Ln = mybir.ActivationFunctionType.Ln
    add = mybir.AluOpType.add
    mult = mybir.AluOpType.mult
    band = mybir.AluOpType.bitwise_and
    is_gt = mybir.AluOpType.is_gt

    nc = bacc.Bacc("TRN2", target_bir_lowering=False, debug=False,
                   enable_asserts=False, num_devices=num_devices)

    x_d = nc.dram_tensor("x", [n_elems], f32, kind="ExternalInput").ap()
    l_d = nc.dram_tensor("lab", [n_elems], i32, kind="ExternalInput").ap()
    patu_d = nc.dram_tensor("patu", [1, C], f32, kind="ExternalInput").ap()
    patl_d = nc.dram_tensor("patl", [1, C], f32, kind="ExternalInput").ap()
    pata_d = nc.dram_tensor("pata", [1, C], f32, kind="ExternalInput").ap()
    out_d = nc.dram_tensor("out", [1, 1], f32, kind="ExternalOutput").ap()

    chunks = _plan_chunks(n_elems, P, F_LOAD)
    slices = _plan_slices(chunks, mix)
    f_alloc = max(f for _, _, f in chunks)
    fs_alloc = max(fs for _, _, fs, _ in slices)
    n_mm_a = repeat * sum(-(-fs // MM_W) for _, _, fs, ty in slices if ty == "A")
    n_mm_m = repeat * sum(-(-fs // MM_W) for _, _, fs, ty in slices if ty == "M")

    with tile.TileContext(nc) as tc, ExitStack() as ctx:
        io = ctx.enter_context(tc.tile_pool(name="io", bufs=io_bufs))
        wk = ctx.enter_context(tc.tile_pool(name="wk", bufs=wk_bufs))
        sg = ctx.enter_context(tc.tile_pool(name="sg", bufs=1))
        ps = ctx.enter_context(tc.tile_pool(name="ps", bufs=1, space="PSUM"))

        ones = sg.tile([P, 1], bf16, tag="ones")
        nc.vector.memset(ones, 1.0)
        patu_t = sg.tile([1, C], f32, tag="patu")
        patl_t = sg.tile([1, C], f32, tag="patl")
        pata_t = sg.tile([1, C], f32, tag="pata")
        # consts ride SWDGE too; they are tiny and go first
        nc.gpsimd.dma_start(out=patu_t, in_=patu_d)
        nc.gpsimd.dma_start(out=patl_t, in_=patl_d)
        nc.gpsimd.dma_start(out=pata_t, in_=pata_d)

        # psum_pad: allocate 512-wide (2048 B = exactly one PSUM bank) so no
        # accumulator straddles a bank boundary
        pw = 512 if psum_pad else MM_W
        pL_full = ps.tile([1, pw], f32, tag="pL", name="pL_full")
        pU_full = ps.tile([1, pw], f32, tag="pU", name="pU_full")
        pA_full = ps.tile([1, pw], f32, tag="pA", name="pA_full")
        pB_full = ps.tile([1, pw], f32, tag="pB", name="pB_full")
        pL = pL_full[:, :MM_W]
        pU = pU_full[:, :MM_W]
        pA = pA_full[:, :MM_W]
        pB = pB_full[:, :MM_W]

        mma = 0
        mmm = 0
        for _ in range(repeat):
            tiles = {}
            for ci, (off, p, f) in enumerate(chunks):
                src_x = x_d[off:off + p * f].rearrange("(p f) -> p f", f=f)
                src_l = l_d[off:off + p * f].rearrange("(p f) -> p f", f=f)
                xt = io.tile([P, f_alloc], bf16, tag="xt", name="xt")
                tb = io.tile([P, f_alloc], bf16, tag="tb", name="tb")
                nc.gpsimd.dma_start(out=xt[:p, :f], in_=src_x)
                nc.gpsimd.dma_start(out=tb[:p, :f], in_=src_l)
                tiles[ci] = (xt, tb, p)
                # compute on the <=F_COMP column slices of this load tile
                for sci, j0, fs, ty in slices:
                    if sci != ci:
                        continue
                    xs = xt[:p, j0:j0 + fs]
                    ts = tb[:p, j0:j0 + fs]
                    ap_t = wk.tile([P, fs_alloc], bf16, tag="ap", name="ap_t")
                    stt_eng = nc.gpsimd if (stt_gpsimd_m and ty == "M") else nc.vector
                    stt_eng.scalar_tensor_tensor(ap_t[:p, :fs], xs, -1.0,
                                                 ts, add, add)
                    if ty == "A":
                        s_t = wk.tile([P, fs_alloc], bf16, tag="w1", name="s_t")
                        L_t = wk.tile([P, fs_alloc], bf16, tag="w2", name="L_t")
                        u_t = wk.tile([P, fs_alloc], bf16, tag="w3", name="u_t")
                        nc.vector.tensor_scalar(s_t[:p, :fs].bitcast(u16),
                                                ap_t[:p, :fs].bitcast(u16),
                                                0x7FFF, None, band)
                        nc.scalar.activation(L_t[:p, :fs], s_t[:p, :fs], Ln)
                        if g_from_sign:
                            # derive t from sign(a') so tb is released right
                            # after the STT (earlier label-DMA buffer reuse)
                            g_t = wk.tile([P, fs_alloc], bf16, tag="w4",
                                          name="g_t")
                            nc.vector.tensor_scalar(g_t[:p, :fs], ap_t[:p, :fs],
                                                    0.0, None, is_gt)
                            nc.vector.tensor_tensor(u_t[:p, :fs], g_t[:p, :fs],
                                                    L_t[:p, :fs], mult)
                        else:
                            nc.vector.tensor_tensor(u_t[:p, :fs], ts,
                                                    L_t[:p, :fs], mult)
                        for j in range(0, fs, MM_W):
                            wd = min(MM_W, fs - j)
                            nc.tensor.matmul(pL[:, :wd], ones[:p, :],
                                             L_t[:p, j:j + wd],
                                             start=(mma == 0),
                                             stop=(mma == n_mm_a - 1))
                            nc.tensor.matmul(pU[:, :wd], ones[:p, :],
                                             u_t[:p, j:j + wd],
                                             start=(mma == 0),
                                             stop=(mma == n_mm_a - 1))
                            mma += 1
                    else:
                        h1_t = wk.tile([P, fs_alloc], bf16, tag="w1", name="h1_t")
                        h2_t = wk.tile([P, fs_alloc], bf16, tag="w2", name="h2_t")
                        u1_t = wk.tile([P, fs_alloc], bf16, tag="w3", name="u1_t")
                        u2_t = wk.tile([P, fs_alloc], bf16, tag="w4", name="u2_t")
                        nc.scalar.activation(h1_t[:p, :fs], ap_t[:p, :fs], Ln)
                        nc.scalar.activation(h2_t[:p, :fs], ap_t[:p, :fs], Ln,
                                             scale=-1.0)
                        nc.vector.tensor_scalar_min(u1_t[:p, :fs],
                                                    h1_t[:p, :fs], 0.0)
                        nc.vector.tensor_scalar_min(u2_t[:p, :fs],
                                                    h2_t[:p, :fs], 0.0)
                        for j in range(0, fs, MM_W):
                            wd = min(MM_W, fs - j)
                            nc.tensor.matmul(pA[:, :wd], ones[:p, :],
                                             u1_t[:p, j:j + wd],
                                             start=(mmm == 0),
                                             stop=(mmm == n_mm_m - 1))
                            nc.tensor.matmul(pB[:, :wd], ones[:p, :],
                                             u2_t[:p, j:j + wd],
                                             start=(mmm == 0),
                                             stop=(mmm == n_mm_m - 1))
                            mmm += 1

        # final fold: strided reduce [1,506]->[1,C], weight, sum -> scalar
        parts = []
        if n_mm_a:
            cu = sg.tile([1, C], f32, tag="cu")
            cl = sg.tile([1, C], f32, tag="cl")
            nc.vector.reduce_sum(cu, pU.rearrange("one (r c) -> one c r", c=C),
                                 axis=mybir.AxisListType.X)
            nc.vector.reduce_sum(cl, pL.rearrange("one (r c) -> one c r", c=C),
                                 axis=mybir.AxisListType.X)
            s1 = sg.tile([1, C], f32, tag="s1")
            s2 = sg.tile([1, C], f32, tag="s2")
            nc.vector.tensor_mul(s1, cu, patu_t)
            nc.vector.tensor_mul(s2, cl, patl_t)
            nc.vector.tensor_add(s1, s1, s2)
            parts.append(s1)
        if n_mm_m:
            ca = sg.tile([1, C], f32, tag="ca")
            cb = sg.tile([1, C], f32, tag="cb")
            nc.vector.reduce_sum(ca, pA.rearrange("one (r c) -> one c r", c=C),
                                 axis=mybir.AxisListType.X)
            nc.vector.reduce_sum(cb, pB.rearrange("one (r c) -> one c r", c=C),
                                 axis=mybir.AxisListType.X)
            s3 = sg.tile([1, C], f32, tag="s3")
            s4 = sg.tile([1, C], f32, tag="s4")
            nc.vector.tensor_mul(s3, ca, pata_t)
            nc.vector.tensor_mul(s4, cb, patl_t)
            nc.vector.tensor_add(s3, s3, s4)
            parts.append(s3)
        tot = parts[0]
        if len(parts) == 2:
            nc.vector.tensor_add(tot, tot, parts[1])
        accf = sg.tile([1, 1], f32, tag="accf")
        nc.vector.reduce_sum(accf, tot, axis=mybir.AxisListType.X)
        nc.sync.dma_start(out=out_d, in_=accf)

    nc.compile()
    return nc


_CACHE = {}


def _get_nc():
    if "nc" not in _CACHE:
        _CACHE["nc"] = build_bass()
    return _CACHE["nc"]


def kernel(x, labels):
    x = np.ascontiguousarray(np.asarray(x, dtype=np.float32))
    labels = np.ascontiguousarray(np.asarray(labels, dtype=np.int32))
    assert x.shape == (B, C), x.shape
    assert labels.shape == (B, C), labels.shape

    nc = _get_nc()
    in_maps = []
    for i in range(N_CORES):
        sl = slice(i * ROWS_PER_CORE, (i + 1) * ROWS_PER_CORE)
        in_maps.append({
            "x": np.ascontiguousarray(x[sl]).reshape(-1),
            "lab": np.ascontiguousarray(labels[sl]).reshape(-1),
            "patu": PATU,
            "patl": PATL,
            "pata": PATA,
        })
    res = bass_utils.run_bass_kernel_spmd(nc, in_maps, core_ids=list(range(N_CORES)))
    total = 0.0
    for r in res.results:
        total += float(r["out"][0, 0])
    return np.float32(total)
